# revision 2
# baseline (speedup 1.0000x reference)
"""Sparse ConvTranspose3d (gather + GEMM + scatter-add) on 8 TRN2 NeuronCores.

v2.1: triple-token scatter (768B descriptors). The kz-fastest offset ordering
makes the 3 output rows for (kx,ky,kz=0,1,2) of one point consecutive after
np.unique's rank compression, so each scatter token covers 3 rows
(elem_size=192, elem_step=64): 3x fewer descriptors than row tokens and above
the 512B HBM small-write threshold.

Within one dma_scatter_add the CCE read-modify-write races between tokens
with overlapping row ranges (cross-instruction overlap is safe; verified on
the baseline). Overlaps only occur between points at sorted-index distance
<=2 (rows per group are strictly increasing), and a z-adjacent point pair
conflicts in all 9 groups at once, so conflict is a per-point property: each
superchunk holds 1024 points = up to 896 "clean" points (slots 0-6, one
scatter instruction per group) plus up to 128 "loser" points (slot 7, a
second small instruction per group). Per-core true token counts ride in a
register (value_load) so trailing -1 index padding generates no descriptors.

Empties are a contiguous tail [u, n_out) written as 128-row bias blocks by
the owning core (other cores aim at scratch rows the host merge never
reads). GEMM runs in bf16 (fp32 PSUM accumulate).
"""
import numpy as np

import bass_rust
import concourse.bass as bass
import concourse.bacc as bacc
import concourse.tile as tile
import concourse.mybir as mybir
from concourse.bass_utils import run_bass_kernel_spmd

N_CORES = 8
KV = 27
NG = 9                            # triple groups (kx,ky)
CIN = 64
COUT = 64
N_OUT = 1620000
SLAB = N_OUT // N_CORES           # 202500
MARGIN = 8192
SC_PTS = 1024                     # points per superchunk (8 chunks of 128)
CLEAN = 896                       # clean-point capacity (slots 0-6)
LOSER = 128                       # loser-point capacity (slot 7)
CPS = SC_PTS // 128               # 8
KAUG = CIN + KV                   # 91 contraction rows (feats + firstmask)
WCOLS = KV * COUT                 # 1728
TB = 32                           # bias-tail rows per token
WIN = 32768

_prog_cache = {}


def _build_program(NSC, bases, work_rows, ntail_tok, tail_base):
    NPTS = NSC * SC_PTS
    tail_elem = TB * COUT
    nc = bacc.Bacc("TRN2", target_bir_lowering=False, debug=False,
                   enable_asserts=False, num_devices=N_CORES,
                   dynamic_dma_scratch_size=65536)
    bf16 = mybir.dt.bfloat16
    ft = nc.dram_tensor("ft", [KAUG, NPTS], bf16, kind="ExternalInput")
    wt = nc.dram_tensor("wt", [KAUG, WCOLS], bf16, kind="ExternalInput")
    bias_in = nc.dram_tensor("bias", [128, tail_elem], mybir.dt.float32,
                             kind="ExternalInput")
    idx = nc.dram_tensor("idx", [NSC, NG, 128, CLEAN // 16], mybir.dt.int16,
                         kind="ExternalInput")
    lidx = nc.dram_tensor("lidx", [NSC, NG, 128, LOSER // 16], mybir.dt.int16,
                          kind="ExternalInput")
    cnts = nc.dram_tensor("cnts", [1, NSC * 2], mybir.dt.int32,
                          kind="ExternalInput")
    tidx = nc.dram_tensor("tidx", [ntail_tok // 128, 128, 8], mybir.dt.int16,
                          kind="ExternalInput")
    work = nc.dram_tensor("work", [work_rows, COUT], mybir.dt.float32,
                          kind="ExternalOutput")
    wtens = work[:].tensor

    with tile.TileContext(nc) as tc:
        with (
            tc.tile_pool(name="const", bufs=1) as cpool,
            tc.tile_pool(name="cbuf", bufs=2) as cbpool,
            tc.tile_pool(name="ipool", bufs=4) as ipool,
            tc.tile_pool(name="psum", bufs=2, space="PSUM") as ppool,
        ):
            ft_t = cpool.tile([KAUG, NPTS], bf16)
            wt_t = cpool.tile([KAUG, WCOLS], bf16)
            cnt_t = cpool.tile([1, NSC * 2], mybir.dt.int32)
            nc.sync.dma_start(out=ft_t[:], in_=ft[:])
            nc.sync.dma_start(out=wt_t[:], in_=wt[:])
            nc.sync.dma_start(out=cnt_t[:], in_=cnts[:])

            # ---- bias tail: scatter 128-row bias blocks (idx in block
            # units; AP base pre-offset to the tail start) ----
            btile = cpool.tile([128, 1, tail_elem], mybir.dt.float32)
            nc.sync.dma_start(out=btile[:],
                              in_=bias_in[:].rearrange("p (a e) -> p a e", a=1))
            tail_ap = bass_rust.AP(
                wtens, int(tail_base) * COUT,
                [[tail_elem, (work_rows - tail_base) * COUT // tail_elem - 1],
                 [1, tail_elem]])
            for j in range(ntail_tok // 128):
                ti_t = ipool.tile([128, 8], mybir.dt.int16)
                nc.sync.dma_start(out=ti_t[:], in_=tidx[j])
                nc.gpsimd.dma_scatter_add(
                    tail_ap, btile[:], ti_t[:], 128, 128, tail_elem,
                    elem_step=tail_elem)

            # ---- main: GEMM + triple-token scatter ----
            for sc in range(NSC):
                c_t = cbpool.tile([128, NG, CPS, 3 * COUT], mybir.dt.float32)
                for ci in range(CPS):
                    ch = sc * CPS + ci
                    ps = ppool.tile([128, WCOLS], mybir.dt.float32,
                                    space="PSUM")
                    for mm in range(4):
                        n0 = mm * 512
                        n1 = min(n0 + 512, WCOLS)
                        nc.tensor.matmul(
                            out=ps[:, n0:n1],
                            lhsT=ft_t[:, ch * 128:(ch + 1) * 128],
                            rhs=wt_t[:, n0:n1],
                            start=True, stop=True)
                    nc.vector.tensor_copy(
                        out=c_t[:, :, ci, :],
                        in_=ps[:].rearrange("p (g e) -> p g e", e=3 * COUT))
                for g in range(NG):
                    base = bases[sc * NG + g]
                    ov = bass_rust.AP(wtens, int(base) * COUT,
                                      [[COUT, 32768], [1, 3 * COUT]])
                    i_t = ipool.tile([128, CLEAN // 16], mybir.dt.int16)
                    nc.sync.dma_start(out=i_t[:], in_=idx[sc, g])
                    nc.gpsimd.dma_scatter_add(
                        ov, c_t[:, g, :CLEAN // 128, :], i_t[:], CLEAN, CLEAN,
                        3 * COUT, elem_step=COUT)
                    li_t = ipool.tile([128, LOSER // 16], mybir.dt.int16)
                    nc.sync.dma_start(out=li_t[:], in_=lidx[sc, g])
                    nc.gpsimd.dma_scatter_add(
                        ov, c_t[:, g, CLEAN // 128:, :], li_t[:], LOSER, LOSER,
                        3 * COUT, elem_step=COUT)
    nc.compile()
    return nc


def _wrap16(vals, cap):
    a = np.full(cap, -1, np.int16)
    a[:len(vals)] = vals
    blk = a.reshape(cap // 16, 16).T
    return np.tile(blk, (8, 1))


def kernel(feats, weight, bias, out_index, n_out):
    feats = np.asarray(feats, np.float32)
    weight = np.asarray(weight, np.float32)
    bias = np.asarray(bias, np.float32)
    oi = np.asarray(out_index, np.int64)
    n_out = int(n_out)

    # ---- sort points spatially; merge duplicate-coordinate points ----
    order = np.argsort(oi[0], kind="stable")
    b0 = oi[0][order]
    dup = np.zeros(len(order), bool)
    dup[1:] = b0[1:] == b0[:-1]
    heads = np.where(~dup, np.arange(len(order)), 0)
    np.maximum.accumulate(heads, out=heads)
    f_s = feats[order].copy()
    if dup.any():
        np.add.at(f_s, heads[dup], f_s[np.flatnonzero(dup)])
    keep = ~dup
    f_s = f_s[keep]
    oi_s = oi[:, order[keep]]                    # [27, M] sorted, deduped
    M = oi_s.shape[1]

    # ---- first-contribution mask (bias exactly once per non-empty row) ----
    u = int(oi_s.max()) + 1                      # inv ids are dense [0, u)
    fm = np.zeros((KV, M), np.float32)
    seen = np.zeros(u, bool)
    for k in range(KV):
        new = ~seen[oi_s[k]]
        fm[k, new] = 1.0
        seen[oi_s[k]] = True

    # ---- assign points to cores by center row ----
    core_of = np.minimum(oi_s[KV // 2] // SLAB, N_CORES - 1)
    work_rows = 2 * MARGIN + SLAB + WIN + 64

    # ---- per-core: mark loser points (scatter-RMW conflicts), pack
    # superchunks as [clean x896 | loser x128] ----
    pts = [np.flatnonzero(core_of == c) for c in range(N_CORES)]
    rows_g = [oi_s[0:KV:3][:, p] - c * SLAB + MARGIN
              for c, p in enumerate(pts)]        # [9, cnt] triple-base rows
    clean_l, loser_l, lsc_l = [], [], []
    max_nsc = 0
    for c in range(N_CORES):
        r = rows_g[c]
        cnt = r.shape[1]
        loser = np.zeros(cnt, bool)
        for i in range(1, cnt):
            for back in (1, 2):
                j = i - back
                if j >= 0 and not loser[j] and \
                        (np.abs(r[:, i] - r[:, j]) <= 2).any():
                    loser[i] = True
                    break
        li = np.flatnonzero(loser)
        # no loser-loser conflicts (would race inside the B instruction)
        for a in range(1, len(li)):
            if li[a] - li[a - 1] <= 2:
                assert not (np.abs(r[:, li[a]] - r[:, li[a - 1]]) <= 2).any()
        ci = np.flatnonzero(~loser)
        clean_l.append(ci)
        loser_l.append(li)
        # superchunk of each clean position; losers join the superchunk of
        # the nearest preceding clean point
        nsc = (len(ci) + CLEAN - 1) // CLEAN
        max_nsc = max(max_nsc, nsc)
        lsc = np.searchsorted(ci, li) - 1
        lsc = np.clip(lsc // CLEAN, 0, nsc - 1)
        assert np.bincount(lsc, minlength=nsc).max() <= LOSER
        lsc_l.append(lsc)
    NSC = max_nsc
    NPTS = NSC * SC_PTS

    # ---- per-(sc, group) window bases over triple base rows ----
    bases = np.zeros(NSC * NG, np.int64)
    for sc in range(NSC):
        for g in range(NG):
            mn, mx = 1 << 60, -1
            for c in range(N_CORES):
                ci = clean_l[c][sc * CLEAN:(sc + 1) * CLEAN]
                segs = [rows_g[c][g, ci]]
                li = loser_l[c][lsc_l[c] == sc]
                if len(li):
                    segs.append(rows_g[c][g, li])
                for seg in segs:
                    if len(seg):
                        mn = min(mn, int(seg.min()))
                        mx = max(mx, int(seg.max()))
            if mx < 0:
                mn = 0
            else:
                assert mx + 2 - mn < 32768, f"span {mx + 2 - mn}"
            bases[sc * NG + g] = mn

    # ---- bias tail: rows [u, n_out) in TB-row blocks on owning core ----
    tail_rows = n_out - u
    nblocks = (tail_rows + TB - 1) // TB
    ntail_tok = ((nblocks + 127) // 128) * 128
    tail_core = min(u // SLAB, N_CORES - 1)
    tail_base = u - tail_core * SLAB + MARGIN    # local row of tail start
    scratch_blk = (SLAB + 2 * MARGIN + TB - 1 - tail_base) // TB + 1
    assert tail_base + (scratch_blk + 1) * TB < work_rows
    assert scratch_blk < 32768 and nblocks <= scratch_blk
    assert tail_base + nblocks * TB <= work_rows

    key = (NSC, tuple(bases), work_rows, ntail_tok, tail_base)
    if key not in _prog_cache:
        _prog_cache[key] = _build_program(NSC, bases, work_rows, ntail_tok,
                                          tail_base)
    nc = _prog_cache[key]

    # ---- per-core input arrays ----
    bf16 = mybir.dt.np(mybir.dt.bfloat16)
    wt_aug = np.zeros((KAUG, WCOLS), np.float32)
    for k in range(KV):
        wt_aug[:CIN, k * COUT:(k + 1) * COUT] = weight[k].T
        wt_aug[CIN + k, k * COUT:(k + 1) * COUT] = bias
    wt_aug = wt_aug.astype(bf16)
    bias_tile = np.tile(bias, (128, TB)).astype(np.float32)
    in_maps = []
    for c in range(N_CORES):
        p, ci_all, li_all, lsc = (pts[c], clean_l[c], loser_l[c], lsc_l[c])
        ft_aug = np.zeros((KAUG, NPTS), np.float32)
        idx_np = np.zeros((NSC, NG, 128, CLEAN // 16), np.int16)
        lidx_np = np.zeros((NSC, NG, 128, LOSER // 16), np.int16)
        cnts_np = np.zeros((1, NSC * 2), np.int32)
        for sc in range(NSC):
            ci = ci_all[sc * CLEAN:(sc + 1) * CLEAN]
            li = li_all[lsc == sc]
            nA, nB = len(ci), len(li)
            cnts_np[0, 2 * sc] = nA
            cnts_np[0, 2 * sc + 1] = nB
            sel = pts[c][ci]
            ft_aug[:CIN, sc * SC_PTS:sc * SC_PTS + nA] = f_s[sel].T
            ft_aug[CIN:, sc * SC_PTS:sc * SC_PTS + nA] = fm[:, sel]
            sell = pts[c][li]
            lo = sc * SC_PTS + CLEAN
            ft_aug[:CIN, lo:lo + nB] = f_s[sell].T
            ft_aug[CIN:, lo:lo + nB] = fm[:, sell]
            for g in range(NG):
                base = bases[sc * NG + g]
                offA = rows_g[c][g, ci] - base
                offB = rows_g[c][g, li] - base
                mx = max(offA.max() if len(offA) else 0,
                         offB.max() if len(offB) else 0)
                pad = mx + 3
                assert pad + 2 < 32768
                fullA = np.full(CLEAN, pad, np.int64)
                fullA[:nA] = offA
                fullB = np.full(LOSER, pad, np.int64)
                fullB[:nB] = offB
                idx_np[sc, g] = _wrap16(fullA.astype(np.int16), CLEAN)
                lidx_np[sc, g] = _wrap16(fullB.astype(np.int16), LOSER)
        tvals = np.full(ntail_tok, scratch_blk, np.int64)
        if c == tail_core:
            tvals[:nblocks] = np.arange(nblocks)
        tidx_np = np.zeros((ntail_tok // 128, 128, 8), np.int16)
        for j in range(ntail_tok // 128):
            tidx_np[j] = _wrap16(tvals[j * 128:(j + 1) * 128].astype(np.int16),
                                 128)
        in_maps.append({"ft": ft_aug.astype(bf16), "wt": wt_aug,
                        "bias": bias_tile, "idx": idx_np, "lidx": lidx_np,
                        "cnts": cnts_np, "tidx": tidx_np})

    res = run_bass_kernel_spmd(nc, in_maps, list(range(N_CORES)))

    # ---- merge halo-overlapped slabs ----
    out = np.zeros((n_out, COUT), np.float32)
    for c in range(N_CORES):
        lo = c * SLAB - MARGIN
        g0, g1 = max(0, lo), min(n_out, (c + 1) * SLAB + MARGIN)
        sl = res.results[c]["work"]
        out[g0:g1] += sl[g0 - lo:g1 - lo]
    return out
